# revision 45
# baseline (speedup 1.0000x reference)
"""Dilated local attention (kernel_size=3, dilation=2) on Trainium2, 8-core SPMD.

Problem: q,k,v [B=4, D=256, N=8192] f32, HEAD_DIM=32 (8 heads), out [B, N, D].
Per position n, head h: logits s_i = SCALE * <q[:,n], k[:,n+2i-2]> for i=0..2
(zero-padded at sequence edges), softmax over {s_0,s_1,s_2} plus six implicit
zero logits (the torch Unfold padding slots), out = sum_i p_i * v[:, n+2i-2].

Sharding: B*H = 32 (batch, head) units; core ci takes 4 units = a contiguous
[128 channel, 8192] block of batch ci//2 (channels (ci%2)*128 ... +128).

Per-core layout: partition p = unit*32 + head_dim, free axis = n.

Structure (v2 — latency-optimized, phases per group):
  Phase A (per 512-slab): DVE q*k_shift products; PE block-ones matmuls ->
    logits L [12, 512]; ACT exp -> E_all slab; PE sum-matmul -> S [4, 512];
    DVE copy packs S into S_sb [4*nslab, 512] (slab s -> partitions 4s..).
  Global (per group): ACT lnZ = Ln(S_sb + 6); ACT R = Exp(-lnZ)  (= 1/Z).
  Phase B (per 512-slab): PE selector matmul broadcasts R -> R12 [12, 512];
    DVE Ppr = E*R12 (normalized probs); PE selector matmuls broadcast to all
    128 partitions; DVE p*v_shift products; PE transpose-matmuls with PSUM
    accumulation sum the 3 taps AND emit the n-major output layout;
    ACT evacuates PSUM->SBUF; one batched DMA stores 512 rows.
"""

import numpy as np

import concourse.bass as bass
import concourse.bacc as bacc
import concourse.mybir as mybir
import concourse.tile as tile
from concourse.bass_utils import run_bass_kernel_spmd

B, D, N = 4, 256, 8192
HD = 32
H = D // HD
SCALE = float(HD) ** -0.5
NCORES = 8
P = 128           # SBUF partitions = 4 units * 32 head dims
UN = P // HD      # units per core
NL = 3 * UN       # logit rows (3 taps * 4 units)
f32 = mybir.dt.float32
AF = mybir.ActivationFunctionType


def _consts(n=N, cs=512, ng=1):
    nslab = n // cs
    sg = nslab // ng          # slabs per group
    sp = UN * sg              # packed S partitions per group
    # lhsT for logit reduction: L[3u+i, n] += SCALE * sum_d P_i[u*32+d, n]
    cl = np.zeros((P, 3 * NL), np.float32)
    for p in range(P):
        u = p // HD
        for i in range(3):
            cl[p, i * NL + 3 * u + i] = SCALE
    # lhsT for group sums: S[u, n] = sum_i E[3u+i, n]
    csum = np.zeros((NL, UN), np.float32)
    for m in range(NL):
        csum[m, m // 3] = 1.0
    # lhsT selecting unit rows from a packed-R block: R12[m, n] = R[m//3, n]
    # (replicated at each 32-aligned partition block so base partitions match)
    crsel = np.zeros((P, NL), np.float32)
    for b in range(4):
        for m in range(NL):
            crsel[32 * b + m // 3, m] = 1.0
    # lhsT for probability broadcast: Pbc_i[p, n] = Ppr[3*(p//32)+i, n]
    csel = np.zeros((NL, 3 * P), np.float32)
    for p in range(P):
        u = p // HD
        for i in range(3):
            csel[3 * u + i, i * P + p] = 1.0
    ident = np.eye(P, dtype=np.float32)
    return cl, csum, crsel, csel, ident


def build_kernel(nc, n=N, ss=1024, cs=512, ng=1, reps=1, sum_on_dve=True,
                 max_nblk=4, scopy_act=True, only_phase=None, dma_bcast=False):
    """Emit the per-core program. n: sequence length; ss: superslab width for
    the q*k products; cs: slab width (512 = one PSUM bank of fp32); ng:
    number of phase groups (1 = single global normalization barrier);
    reps>1 wraps everything (incl. input DMA) in an on-device benchmark loop."""
    assert ss % cs == 0 and n % ss == 0 and cs % P == 0
    nslab = n // cs
    assert nslab % ng == 0
    sg = nslab // ng          # slabs per group
    nblk = min(max_nblk, sg)  # 32-aligned partition blocks used in packed S
    nchunk = (sg + nblk - 1) // nblk
    assert nchunk * nblk == sg

    q_d = nc.declare_dram_parameter("q", [P, n], f32, isOutput=False)
    k_d = nc.declare_dram_parameter("k", [P, n], f32, isOutput=False)
    v_d = nc.declare_dram_parameter("v", [P, n], f32, isOutput=False)
    cl_d = nc.declare_dram_parameter("cl", [P, 3 * NL], f32, isOutput=False)
    csum_d = nc.declare_dram_parameter("csum", [NL, UN], f32, isOutput=False)
    crsel_d = nc.declare_dram_parameter("crsel", [P, NL], f32, isOutput=False)
    csel_d = nc.declare_dram_parameter("csel", [NL, 3 * P], f32, isOutput=False)
    ident_d = nc.declare_dram_parameter("ident", [P, P], f32, isOutput=False)
    out_d = nc.declare_dram_parameter("out", [n, P], f32, isOutput=True)

    ldc = 2048 if n % 2048 == 0 else cs  # input DMA chunk

    with tile.TileContext(nc) as tc:
        with (
            tc.tile_pool(name="const", bufs=1) as const_pool,
            tc.tile_pool(name="big", bufs=1) as big_pool,
            tc.tile_pool(name="pprod", bufs=2) as p_pool,
            tc.tile_pool(name="tprod", bufs=2) as t_pool,
            tc.tile_pool(name="soft", bufs=4) as sm_pool,
            tc.tile_pool(name="outsb", bufs=3) as o_pool,
            tc.tile_pool(name="psL", bufs=3 if dma_bcast else 2,
                         space="PSUM") as psL,
            tc.tile_pool(name="psS", bufs=1, space="PSUM") as psS,
            tc.tile_pool(name="psR", bufs=1, space="PSUM") as psR,
            tc.tile_pool(name="psP", bufs=1, space="PSUM") as psP,
            tc.tile_pool(name="psO", bufs=3 if dma_bcast else 1,
                         space="PSUM") as psO,
        ):
            cl_t = const_pool.tile([P, 3 * NL], f32)
            nc.sync.dma_start(out=cl_t[:], in_=cl_d[:])
            csum_t = const_pool.tile([NL, UN], f32)
            nc.sync.dma_start(out=csum_t[:], in_=csum_d[:])
            crsel_t = const_pool.tile([P, NL], f32)
            nc.sync.dma_start(out=crsel_t[:], in_=crsel_d[:])
            csel_t = const_pool.tile([NL, 3 * P], f32)
            nc.sync.dma_start(out=csel_t[:], in_=csel_d[:])
            ident_t = const_pool.tile([P, P], f32)
            nc.sync.dma_start(out=ident_t[:], in_=ident_d[:])
            bias6 = const_pool.tile([P, 1], f32)
            nc.gpsimd.memset(bias6[:], 6.0)

            qb = big_pool.tile([P, n], f32)
            kb = big_pool.tile([P, n + 4], f32)
            vb = big_pool.tile([P, n + 4], f32)
            E_all = big_pool.tile([NL, n], f32)
            nc.gpsimd.memset(kb[:, 0:2], 0.0)
            nc.gpsimd.memset(kb[:, n + 2 : n + 4], 0.0)
            nc.gpsimd.memset(vb[:, 0:2], 0.0)
            nc.gpsimd.memset(vb[:, n + 2 : n + 4], 0.0)

            def body():
                for c in range(n // ldc):
                    sl = slice(c * ldc, (c + 1) * ldc)
                    sl2 = slice(2 + c * ldc, 2 + (c + 1) * ldc)
                    nc.sync.dma_start(out=qb[:, sl], in_=q_d[:, sl])
                    nc.sync.dma_start(out=kb[:, sl2], in_=k_d[:, sl])
                    nc.sync.dma_start(out=vb[:, sl2], in_=v_d[:, sl])
                for g in range(ng):
                    group(g)

            def group(g):
                s0 = g * sg  # first slab of group
                # ---- Phase A ----
                # packed S: slab s -> partitions 32*(s%nblk)..+4, free chunk s//nblk
                S_sb = sm_pool.tile([32 * (nblk - 1) + UN, nchunk * cs], f32,
                                    name="S_sb", bufs=1)
                if only_phase != "B":
                    if nblk > 1:
                        nc.gpsimd.memset(S_sb[:], 1.0)  # fill inter-block holes
                    for s in range(s0, s0 + sg):
                        if (s * cs) % ss == 0:
                            phaseA_products(s * cs)
                        phaseA_slab(s, s - s0, S_sb)
                    # ---- Global normalization (1/Z of the group, in place) ----
                    nc.scalar.activation(S_sb[:], S_sb[:], AF.Ln,
                                         bias=bias6[0 : 32 * (nblk - 1) + UN, :])
                    nc.scalar.activation(S_sb[:], S_sb[:], AF.Exp, scale=-1.0)
                elif g == 0:
                    nc.gpsimd.memset(S_sb[:], 1.0)
                    nc.gpsimd.memset(E_all[:], 0.5)
                # ---- Phase B ----
                if only_phase != "A":
                    for s in range(s0, s0 + sg):
                        phaseB_slab(s, s - s0, S_sb)

            def phaseA_products(n0):
                Pall = p_pool.tile([P, 3 * ss], f32, name="Pall")
                for i in range(3):
                    nc.vector.tensor_mul(
                        Pall[:, i * ss : (i + 1) * ss],
                        qb[:, n0 : n0 + ss],
                        kb[:, n0 + 2 * i : n0 + 2 * i + ss],
                    )
                phaseA_products.cur = (n0, Pall)

            def phaseA_slab(s, sl, S_sb):
                n0 = s * cs
                p0, Pall = phaseA_products.cur
                off = n0 - p0
                L = psL.tile([NL, cs], f32, name="L")
                for i in range(3):
                    nc.tensor.matmul(
                        L[:],
                        cl_t[:, i * NL : (i + 1) * NL],
                        Pall[:, i * ss + off : i * ss + off + cs],
                        start=(i == 0),
                        stop=(i == 2),
                    )
                nc.scalar.activation(E_all[:, n0 : n0 + cs], L[:], AF.Exp)
                S = psS.tile([UN, cs], f32, name="S")
                nc.tensor.matmul(
                    S[:], csum_t[:], E_all[:, n0 : n0 + cs], start=True, stop=True
                )
                pb = 32 * (sl % nblk)
                fc = (sl // nblk) * cs
                if scopy_act:
                    nc.scalar.copy(S_sb[pb : pb + UN, fc : fc + cs], S[:])
                else:
                    nc.vector.tensor_copy(S_sb[pb : pb + UN, fc : fc + cs], S[:])

            def phaseB_slab(s, sl, R_sb):
                n0 = s * cs
                pb = 32 * (sl % nblk)
                fc = (sl // nblk) * cs
                Ppr = sm_pool.tile([NL, cs], f32, name="Ppr")
                if dma_bcast:
                    # broadcast 1/Z rows to the 12 logit rows via DMA (stride-0)
                    R12 = sm_pool.tile([NL, cs], f32, name="R12")
                    nc.sync.dma_start(
                        out=R12[:].rearrange("(u i) n -> u i n", u=UN),
                        in_=R_sb[pb : pb + UN, fc : fc + cs][:, None, :]
                        .broadcast_to([UN, 3, cs]),
                    )
                    nc.vector.tensor_mul(Ppr[:], E_all[:, n0 : n0 + cs], R12[:])
                    # broadcast probs to all 128 partitions via DMA
                    Pbc = t_pool.tile([P, 3 * cs], f32, name="Pbc")
                    pprv = Ppr[:].rearrange("(u i) n -> u i n", u=UN)
                    for i in range(3):
                        nc.sync.dma_start(
                            out=Pbc[:, i * cs : (i + 1) * cs].rearrange(
                                "(u d) n -> u d n", u=UN
                            ),
                            in_=pprv[:, i, :][:, None, :]
                            .broadcast_to([UN, HD, cs]),
                        )
                else:
                    R12 = psR.tile([NL, cs], f32, name="R12ps")
                    nc.tensor.matmul(
                        R12[:],
                        crsel_t[pb : pb + UN, :],
                        R_sb[pb : pb + UN, fc : fc + cs],
                        start=True,
                        stop=True,
                        tile_position=(pb, 0),
                    )
                    nc.vector.tensor_mul(Ppr[:], E_all[:, n0 : n0 + cs], R12[:])
                    Pbc = psP.tile([P, 3 * cs], f32, name="Pbc")
                    for i in range(3):
                        nc.tensor.matmul(
                            Pbc[:, i * cs : (i + 1) * cs],
                            csel_t[:, i * P : (i + 1) * P],
                            Ppr[:],
                            start=True,
                            stop=True,
                        )
                T = t_pool.tile([P, 3 * cs], f32, name="T")
                for i in range(3):
                    nc.vector.tensor_mul(
                        T[:, i * cs : (i + 1) * cs],
                        Pbc[:, i * cs : (i + 1) * cs],
                        vb[:, n0 + 2 * i : n0 + 2 * i + cs],
                    )
                O = psO.tile([P, cs], f32, name="O")
                if sum_on_dve:
                    nc.vector.tensor_add(T[:, 0:cs], T[:, 0:cs], T[:, cs : 2 * cs])
                    nc.vector.tensor_add(T[:, 0:cs], T[:, 0:cs], T[:, 2 * cs : 3 * cs])
                    for cc in range(cs // P):
                        nc.tensor.matmul(
                            O[:, cc * P : (cc + 1) * P],
                            T[:, cc * P : cc * P + P],
                            ident_t[:],
                            is_transpose=True,
                            start=True,
                            stop=True,
                        )
                else:
                    for cc in range(cs // P):
                        for i in range(3):
                            nc.tensor.matmul(
                                O[:, cc * P : (cc + 1) * P],
                                T[:, i * cs + cc * P : i * cs + cc * P + P],
                                ident_t[:],
                                is_transpose=True,
                                start=(i == 0),
                                stop=(i == 2),
                            )
                osb = o_pool.tile([P, cs], f32, name="osb")
                nc.scalar.copy(osb[:], O[:])
                nc.sync.dma_start(
                    out=out_d[n0 : n0 + cs, :].rearrange("(cc p) col -> p cc col", p=P),
                    in_=osb[:].rearrange("p (cc col) -> p cc col", cc=cs // P),
                )

            if reps == 1:
                body()
            else:
                with tc.For_i(0, reps, 1):
                    body()
    return nc


_cache = {}


def _get_nc():
    if "nc" not in _cache:
        nc = bacc.Bacc(None, target_bir_lowering=False, debug=False)
        build_kernel(nc)
        nc.compile()
        _cache["nc"] = nc
    return _cache["nc"]


def make_in_maps(q, k, v, n=N, cs=512, ng=1):
    cl, csum, crsel, csel, ident = _consts(n=n, cs=cs, ng=ng)
    in_maps = []
    for ci in range(NCORES):
        b = ci // 2
        c0 = (ci % 2) * P
        in_maps.append(
            {
                "q": np.ascontiguousarray(q[b, c0 : c0 + P, :]),
                "k": np.ascontiguousarray(k[b, c0 : c0 + P, :]),
                "v": np.ascontiguousarray(v[b, c0 : c0 + P, :]),
                "cl": cl,
                "csum": csum,
                "crsel": crsel,
                "csel": csel,
                "ident": ident,
            }
        )
    return in_maps


def run_sharded(q, k, v, **spmd_kwargs):
    q = np.ascontiguousarray(np.asarray(q), dtype=np.float32)
    k = np.ascontiguousarray(np.asarray(k), dtype=np.float32)
    v = np.ascontiguousarray(np.asarray(v), dtype=np.float32)
    assert q.shape == (B, D, N)
    nc = _get_nc()
    in_maps = make_in_maps(q, k, v)
    res = run_bass_kernel_spmd(nc, in_maps, list(range(NCORES)), **spmd_kwargs)
    out = np.empty((B, N, D), np.float32)
    for ci, r in enumerate(res.results):
        b = ci // 2
        c0 = (ci % 2) * P
        out[b, :, c0 : c0 + P] = r["out"]
    return out, res


def kernel(q, k, v):
    return run_sharded(q, k, v)[0]
